# revision 16
# baseline (speedup 1.0000x reference)
"""ColorHistogramLoss Trainium2 kernel (8 NeuronCores, data-parallel).

Strategy: shard batch (32 -> 4 per core); each core streams its 8 images
(4 real + 4 fake) as [128, 2048] plane-triples and produces direct
cumulative histogram-edge counts; the host reassembles the three 10-bin
histograms per source and computes the scalar loss.

v2.2 design (VectorE+ScalarE balanced; GPSIMD unusable - SBUF port
contention with VectorE inflates both ~2.8x).  Histograms are per-SOURCE
(sum over 4 images), so counting accumulates across images: images are
processed in PAIRS with [128, 2, 2048] tiles - per-image tensor_tensor
ops write slices, everything downstream (fused customs, counting, Sign
activations) runs once per pair at FD=4096, halving instruction count
and amortizing fixed per-op costs.

- VectorE per pair: per-image v=b-r, w=r-g, u=g-b, m1, mx (10 TTs at
  FD=2048) + pair-level fused customs d=max(|v|,|w|,|v+w|),
  s8=mb+mg from (v,w) (8 ALU slices), rd=1/d, u'=u+BIG*s8 (in-place),
  w'=w+BIG*(1-mb) (in-place), v'=v+BIG*(1-mg) (in-place), q=mx*rd,
  then 6 dual-count passes: hue edges as dual multiply-compares
  (u'|v'|w')*rd vs case edges (out-of-case pixels sit at >= BIG-1,
  outside every edge, so counts are exactly per-case), one mixed dual
  {w'rd<0.8}+PACK*{w'<2} (B-case total rides free), one val dual.
- ScalarE per pair: 18 Sign activations with accum: val mx @
  {0.1,0.2,0.5..0.9}, sat q @ {10/k} (sat<c <=> q>10/k), NEG=#{u'<0},
  G-total=#{v'<2}.

Counts are exact in f32 (recip approx gives ~1e-7 slivers only); host
decode uses direct per-case cumulative counts.
"""

import sys

if "/opt/trn_rl_repo" not in sys.path:
    sys.path.insert(0, "/opt/trn_rl_repo")

import numpy as np

from concourse import bacc, mybir, tile
from concourse import bass_utils

# ---- problem constants (hardcoded; kernel.py must be self-contained) ----
B, C, H, W = 32, 3, 512, 512
NCORES = 8
BPC = B // NCORES            # batches per core
P, F = 128, 2048             # SBUF tile: one [512,512] plane = [128, 2048]
NITER = 2 * BPC              # 8 images per core
NPAIR = NITER // 2           # 4 pair-iterations (2 real, 2 fake)
ACCW = 32                    # padded accumulator width
NPIX = B * H * W             # pixels per full histogram (all cores)
ALPHA, BETA, GAMMA = 0.3, 0.4, 0.4

AF = mybir.AluOpType
F32 = mybir.dt.float32

LAST_EXEC_NS = None
_CACHE = {}

PACK = 4096.0   # dual-count packing: accum = cnt_lo + PACK*cnt_hi (exact f32)
BIG = 1048576.0  # out-of-case shift; BIG*rd >= BIG-1 >> all hue edges

# ScalarE sign-counted edges
S_MX = (0.1, 0.2, 0.5, 0.6, 0.7, 0.8, 0.9)   # val: count mx < e (slots 10-16)
S_Q = tuple(10.0 / k for k in range(1, 10))  # sat: count q > e   (slots 17-25)

NSLOT = 26


def _register_custom_ops():
    """Author + register fused DVE ops in the dve_ops registry at runtime."""
    from concourse import dve_ops
    from concourse.dve_spec import (
        C0, C1, C2, One, Spec, Src0, Src1, Zero, _has_src1, lower, maxx, minn,
    )
    from concourse.dve_uop import DveOpSpec
    from operator import add as _add

    if hasattr(dve_ops, "D3X"):
        return dve_ops

    def cref(f):
        def r(in0, in1, c0, c1, c2):
            b = f(in0, in1, c0, c1, c2).astype(np.float32)
            return b, b.reshape(b.shape[0], -1).sum(-1, keepdims=True)
        return r

    _t = Src0 + Src1

    def _mg(v, w):
        return ((v + w) < 0) & (w <= 0)

    def _mb(v, w):
        return (v >= 0) & ((v + w) >= 0)

    defs = [
        # d = max(|v|,|w|,|v+w|) == mx-mn (exact: same fl subtract results)
        ("D3X", Spec(
            body=maxx(maxx(maxx(Src0, Src1), _t), Zero - minn(minn(Src0, Src1), _t)),
            reference=lambda in0, in1, c0, c1, c2: np.maximum(
                np.maximum(np.abs(in0), np.abs(in1)), np.abs(in0 + in1)
            ).astype(np.float32),
        )),
        # s8 = mb + mg from (v,w): mb=(v>=0)&(v+w>=0), mg=(v+w<0)&(w<=0)
        ("S8C", Spec(
            body=((Src0 >= Zero) & (_t >= Zero)) + ((_t < Zero) & (Src1 <= Zero)),
            reference=lambda in0, in1, c0, c1, c2: (
                _mb(in0, in1) + _mg(in0, in1)
            ).astype(np.float32),
        )),
        # v' = v + C0*(1 - mg) with mg=(u>0)&(u+v>=0)   (Src0=u, Src1=v)
        ("VSH2", Spec(
            body=Src1 + C0 * (One - ((Src0 > Zero) & (_t >= Zero))),
            reference=lambda in0, in1, c0, c1, c2: (
                in1 + c0 * (1.0 - ((in0 > 0) & ((in0 + in1) >= 0)))
            ).astype(np.float32),
        )),
        # w' = w + C0*(1 - mb) with mb=(u+w<=0)&(u<=0)  (Src0=u, Src1=w)
        ("WSH2", Spec(
            body=Src1 + C0 * (One - ((_t <= Zero) & (Src0 <= Zero))),
            reference=lambda in0, in1, c0, c1, c2: (
                in1 + c0 * (1.0 - (((in0 + in1) <= 0) & (in0 <= 0)))
            ).astype(np.float32),
        )),
        # dual mult-compare count: accum = #{a*b < C0} + C1*#{a*b < C2}
        ("MULCMP", Spec(
            body=(Src0 * Src1 < C0) + C1 * ((Src0 * Src1) < C2),
            accum=_add, accum_init=Zero,
            reference=cref(lambda a, b, c0, c1, c2: (
                ((a * b).astype(np.float32) < c0)
                + c1 * ((a * b).astype(np.float32) < c2)
            )),
        )),
        # mixed dual count: accum = #{a*b < C0} + C1*#{a < C2}
        ("MULCMPM", Spec(
            body=(Src0 * Src1 < C0) + C1 * (Src0 < C2),
            accum=_add, accum_init=Zero,
            reference=cref(lambda a, b, c0, c1, c2: (
                ((a * b).astype(np.float32) < c0) + c1 * (a < c2)
            )),
        )),
        # dual edge count: accum = #{x < C0} + C1*#{x < C2}
        ("LT2", Spec(
            body=(Src0 < C0) + C1 * (Src0 < C2),
            accum=_add, accum_init=Zero,
            reference=cref(lambda a, b, c0, c1, c2: (a < c0) + c1 * (a < c2)),
        )),
    ]
    for name, spec in defs:
        row = 1 + len(dve_ops.OPS)
        shas = {}
        for ver in ("v3", "v4"):
            uops = lower(spec, ver=ver)
            shas[ver] = DveOpSpec(
                name=name, opcode=row, uops=uops, rd1_en=_has_src1(spec)
            ).sha(ver)
        op = dve_ops.DveOp(name, spec, False, uops_sha=shas)
        dve_ops.OPS.append(op)
        dve_ops.CUSTOM_DVE_SPECS[name] = spec
        dve_ops._SUB_OPCODE_FOR_NAME[name] = row
        setattr(dve_ops, name, op)
    return dve_ops


def _build():
    dve_ops = _register_custom_ops()
    nc = bacc.Bacc(
        "TRN2", target_bir_lowering=False, debug=False, num_devices=NCORES
    )
    xr = nc.dram_tensor("x_real", [BPC * C * P, F], F32, kind="ExternalInput").ap()
    xf = nc.dram_tensor("x_fake", [BPC * C * P, F], F32, kind="ExternalInput").ap()
    out = nc.dram_tensor("out", [NPAIR * P, ACCW], F32, kind="ExternalOutput").ap()

    SIGN = mybir.ActivationFunctionType.Sign

    with tile.TileContext(nc) as tc:
        with tc.tile_pool(name="main", bufs=2) as io_pool, tc.tile_pool(
            name="tmp", bufs=1
        ) as tmp_pool:
            V, S = nc.vector, nc.scalar
            # bias tiles for ScalarE sign counts (bias = -edge)
            sbias = []
            for idx, e in enumerate(S_MX + S_Q + (0.0, 2.0)):
                bt = tmp_pool.tile([P, 1], F32, tag=f"sb{idx}", name=f"sb{idx}")
                nc.gpsimd.memset(bt[:], -e)
                sbias.append(bt)
            b_mx = sbias[0:7]
            b_q = sbias[7:16]
            b_0, b_2 = sbias[16], sbias[17]

            for pr in range(NPAIR):
                src = xr if pr < NPAIR // 2 else xf

                def plane(j, c):
                    qI = ((pr * 2 + j) % BPC) * C + c
                    return src[qI * P : (qI + 1) * P, :]

                def TP(tag):  # pair tile [P, 2*F] (two images side by side)
                    return tmp_pool.tile([P, 2 * F], F32, tag=tag, name=tag)

                acc = io_pool.tile([P, NSLOT], F32, tag="acc")
                vP = TP("vP")
                wP = TP("wP")
                uP = TP("uP")
                mxP = TP("mxP")
                qP = TP("qP")
                rdP = TP("rdP")
                s8P = TP("s8P")
                scrP = TP("scrP")   # doubles as d
                scr2 = tmp_pool.tile(
                    [P, 2 * F], mybir.dt.bfloat16, tag="scr2", name="scr2"
                )
                m1 = tmp_pool.tile([P, F], F32, tag="m1", name="m1", bufs=2)

                for j in range(2):
                    r = io_pool.tile([P, F], F32, tag="r", name="r")
                    g = io_pool.tile([P, F], F32, tag="g", name="g")
                    bl = io_pool.tile([P, F], F32, tag="bl", name="bl")
                    nc.sync.dma_start(bl[:], plane(j, 2))
                    nc.sync.dma_start(r[:], plane(j, 0))
                    nc.sync.dma_start(g[:], plane(j, 1))
                    sl = slice(j * F, (j + 1) * F)
                    V.tensor_tensor(vP[:, sl], bl[:], r[:], AF.subtract)
                    V.tensor_tensor(wP[:, sl], r[:], g[:], AF.subtract)
                    V.tensor_tensor(m1[:], r[:], g[:], AF.max)
                    V.tensor_tensor(uP[:, sl], g[:], bl[:], AF.subtract)
                    V.tensor_tensor(mxP[:, sl], m1[:], bl[:], AF.max)

                # pair-level fused ops
                V._custom_dve(dve_ops.D3X, out=scrP[:], in0=vP[:], in1=wP[:])
                V.reciprocal_approx_fast(rdP[:], scrP[:])
                V._custom_dve(dve_ops.S8C, out=s8P[:], in0=vP[:], in1=wP[:])
                # in-place shifts; masks recomputed from original u, which is
                # only overwritten by the final STT
                V._custom_dve(dve_ops.WSH2, out=wP[:], in0=uP[:], in1=wP[:], s0=BIG)
                V._custom_dve(dve_ops.VSH2, out=vP[:], in0=uP[:], in1=vP[:], s0=BIG)
                V.scalar_tensor_tensor(uP[:], s8P[:], BIG, uP[:], AF.mult, AF.add)
                V.tensor_tensor(qP[:], mxP[:], rdP[:], AF.mult)

                # --- ScalarE sign counts (mx first, then u'/v', then q) ---
                for k in range(7):
                    S.activation(scr2[:], mxP[:], SIGN, bias=b_mx[k][:],
                                 accum_out=acc[:, 10 + k : 11 + k])

                # --- VectorE dual counts (pair-level) ---
                def mc(op, src0, src1, e1, e2, slot):
                    V._custom_dve(op, out=scrP[:], in0=src0[:], in1=src1[:],
                                  s0=e1, s1=PACK, imm2=e2,
                                  accum_out=acc[:, slot : slot + 1])

                mc(dve_ops.MULCMP, uP, rdP, -0.6, 0.6, 0)
                mc(dve_ops.MULCMP, vP, rdP, -0.8, -0.2, 1)
                mc(dve_ops.MULCMP, vP, rdP, 0.4, 1.0, 2)
                mc(dve_ops.MULCMP, wP, rdP, -0.4, 0.2, 3)
                # slot4: {w'rd < 0.8} + PACK*{w' < 2}  (NB(0.8) + Bt)
                mc(dve_ops.MULCMPM, wP, rdP, 0.8, 2.0, 4)
                V._custom_dve(dve_ops.LT2, out=scrP[:], in0=mxP[:], s0=0.3,
                              s1=PACK, imm2=0.4, accum_out=acc[:, 5:6])

                # --- remaining ScalarE sign counts ---
                S.activation(scr2[:], uP[:], SIGN, bias=b_0[:],
                             accum_out=acc[:, 7:8])
                S.activation(scr2[:], vP[:], SIGN, bias=b_2[:],
                             accum_out=acc[:, 8:9])
                for k in range(9):
                    S.activation(scr2[:], qP[:], SIGN, bias=b_q[k][:],
                                 accum_out=acc[:, 17 + k : 18 + k])

                nc.sync.dma_start(out[pr * P : (pr + 1) * P, 0:NSLOT], acc[:, :])

    nc.compile()
    return nc


def _register_ntff_hook():
    """Register the axon NTFF profiling hook; keep artifacts local."""
    import types

    import antenv

    if "antenv.axon_hooks" not in sys.modules:
        mod = types.ModuleType("antenv.axon_hooks")
        holder = [None]
        mod.set_axon_ntff_profile_hook = lambda h: holder.__setitem__(0, h)
        mod.get_axon_ntff_profile_hook = lambda: holder[0]
        sys.modules["antenv.axon_hooks"] = mod
        antenv.axon_hooks = mod
    from antenv import axon_hooks

    if axon_hooks.get_axon_ntff_profile_hook() is None:
        from trn_agent_boot.trn_boot import _ntff_profile_via_ctypes

        axon_hooks.set_axon_ntff_profile_hook(
            _ntff_profile_via_ctypes("/opt/axon/libaxon_pjrt.so")
        )
    bass_utils.upload_artifacts = lambda tmpdir: tmpdir


def _get_nc():
    if "nc" not in _CACHE:
        _CACHE["nc"] = _build()
    return _CACHE["nc"]


def kernel(x_real: np.ndarray, x_fake: np.ndarray) -> np.ndarray:
    global LAST_EXEC_NS
    nc = _get_nc()

    in_maps = []
    for c in range(NCORES):
        sl = slice(c * BPC, (c + 1) * BPC)
        in_maps.append(
            {
                "x_real": np.ascontiguousarray(x_real[sl]).reshape(BPC * C * P, F),
                "x_fake": np.ascontiguousarray(x_fake[sl]).reshape(BPC * C * P, F),
            }
        )

    import os

    trace = bool(int(os.environ.get("KERNEL_TRACE", "0")))
    if trace:
        _register_ntff_hook()
    res = bass_utils.run_bass_kernel_spmd(
        nc, in_maps, core_ids=list(range(NCORES)), trace=trace
    )
    LAST_EXEC_NS = res.exec_time_ns
    _CACHE["last_res"] = res

    # ---- host decode ----
    # slots: 0 u'rd dual(-0.6,0.6) | 1 v'rd dual(-0.8,-0.2) | 2 v'rd dual(0.4,1.0)
    #        3 w'rd dual(-0.4,0.2) | 4 {w'rd<0.8}+P*{w'<2} | 5 mx dual(0.3,0.4)
    #        7 sign(u') | 8 sign(v'-2) | 10-16 sign(mx-e), e in S_MX
    #        17-25 sign(q-10/k), k=1..9
    slots = np.zeros((2, NSLOT), np.float64)
    duals = np.zeros((2, 6, 2), np.float64)  # per-row unpacked dual slots 0..5
    half = NPAIR // 2
    for core_out in res.results:
        o = np.asarray(core_out["out"]).reshape(NPAIR, P, ACCW)[:, :, :NSLOT]
        slots[0] += o[:half].sum(axis=(0, 1))
        slots[1] += o[half:].sum(axis=(0, 1))
        pk = o[:, :, 0:6].astype(np.int64)  # exact ints in f32
        lo, hi = pk % int(PACK), pk // int(PACK)
        for t_idx, sl in ((0, slice(0, half)), (1, slice(half, NPAIR))):
            duals[t_idx, :, 0] += lo[sl].sum(axis=(0, 1))
            duals[t_idx, :, 1] += hi[sl].sum(axis=(0, 1))

    N = float(NPIX)  # pixels per source across all cores

    C_lt = np.zeros((2, 3, 9), np.float64)
    for t in range(2):
        NA_lo, NA_hi = duals[t, 0]                 # NA(-0.6), NA(0.6)
        NG_m8, NG_m2 = duals[t, 1]
        NG_04, NG_10 = duals[t, 2]
        NB_m4, NB_02 = duals[t, 3]
        NB_08, Bt = duals[t, 4]                    # NB(0.8), caseB total
        NEG = (N - slots[t, 7]) / 2.0              # #{u' < 0}
        Gt = (N - slots[t, 8]) / 2.0               # caseG total
        R = N - Gt - Bt
        # hue cumulative at 0.6k, k=1..9
        C_lt[t, 0, 0] = NA_hi - NEG
        for j, ng in enumerate((NG_m8, NG_m2, NG_04, NG_10)):
            C_lt[t, 0, 1 + j] = (R - NEG) + ng
        for j, nb in enumerate((NB_m4, NB_02, NB_08)):
            C_lt[t, 0, 5 + j] = (R - NEG) + Gt + nb
        C_lt[t, 0, 8] = (N - NEG) + NA_lo
        # val cumulative at 0.1k: 0.1,0.2 signs; 0.3,0.4 dual; 0.5-0.9 signs
        C_lt[t, 2, 0] = (N - slots[t, 10]) / 2.0
        C_lt[t, 2, 1] = (N - slots[t, 11]) / 2.0
        C_lt[t, 2, 2:4] = duals[t, 5]
        for k in range(5):
            C_lt[t, 2, 4 + k] = (N - slots[t, 12 + k]) / 2.0
        # sat cumulative: C(0.1k) = #{q > 10/k} = (N + sign_sum)/2
        for k in range(9):
            C_lt[t, 1, k] = (N + slots[t, 17 + k]) / 2.0

    hist = np.zeros((2, 3, 10), np.float64)
    hist[:, :, 0] = C_lt[:, :, 0]
    hist[:, :, 1:9] = C_lt[:, :, 1:] - C_lt[:, :, :-1]
    hist[:, :, 9] = N - C_lt[:, :, 8]

    dmean = np.abs(hist[0] - hist[1]).mean(axis=1)   # [3] = h, s, v
    loss = ALPHA * dmean[0] + BETA * dmean[1] + GAMMA * dmean[2]
    return np.asarray(loss, dtype=np.float32)


# revision 18
# speedup vs baseline: 1.0352x; 1.0352x over previous
"""ColorHistogramLoss Trainium2 kernel (8 NeuronCores, data-parallel).

Strategy: shard batch (32 -> 4 per core); each core streams its 8 images
(4 real + 4 fake) as [128, 2048] plane-triples and produces direct
cumulative histogram-edge counts; the host reassembles the three 10-bin
histograms per source and computes the scalar loss.

v2.2 design (VectorE+ScalarE balanced; GPSIMD unusable - SBUF port
contention with VectorE inflates both ~2.8x).  Histograms are per-SOURCE
(sum over 4 images), so counting accumulates across images: images are
processed in PAIRS with [128, 2, 2048] tiles - per-image tensor_tensor
ops write slices, everything downstream (fused customs, counting, Sign
activations) runs once per pair at FD=4096, halving instruction count
and amortizing fixed per-op costs.

- VectorE per pair: per-image v=b-r, w=r-g, u=g-b, m1, mx (10 TTs at
  FD=2048) + pair-level fused customs d=max(|v|,|w|,|v+w|),
  s8=mb+mg from (v,w) (8 ALU slices), rd=1/d, u'=u+BIG*s8 (in-place),
  w'=w+BIG*(1-mb) (in-place), v'=v+BIG*(1-mg) (in-place), q=mx*rd,
  then 6 dual-count passes: hue edges as dual multiply-compares
  (u'|v'|w')*rd vs case edges (out-of-case pixels sit at >= BIG-1,
  outside every edge, so counts are exactly per-case), one mixed dual
  {w'rd<0.8}+PACK*{w'<2} (B-case total rides free), one val dual.
- ScalarE per pair: 18 Sign activations with accum: val mx @
  {0.1,0.2,0.5..0.9}, sat q @ {10/k} (sat<c <=> q>10/k), NEG=#{u'<0},
  G-total=#{v'<2}.

Counts are exact in f32 (recip approx gives ~1e-7 slivers only); host
decode uses direct per-case cumulative counts.
"""

import sys

if "/opt/trn_rl_repo" not in sys.path:
    sys.path.insert(0, "/opt/trn_rl_repo")

import numpy as np

from concourse import bacc, mybir, tile
from concourse import bass_utils

# ---- problem constants (hardcoded; kernel.py must be self-contained) ----
B, C, H, W = 32, 3, 512, 512
NCORES = 8
BPC = B // NCORES            # batches per core
P, F = 128, 2048             # SBUF tile: one [512,512] plane = [128, 2048]
NITER = 2 * BPC              # 8 images per core
NPAIR = NITER // 2           # 4 pair-iterations (2 real, 2 fake)
ACCW = 32                    # padded accumulator width
NPIX = B * H * W             # pixels per full histogram (all cores)
ALPHA, BETA, GAMMA = 0.3, 0.4, 0.4

AF = mybir.AluOpType
F32 = mybir.dt.float32

LAST_EXEC_NS = None
_CACHE = {}

PACK = 4096.0   # dual-count packing: accum = cnt_lo + PACK*cnt_hi (exact f32)
BIG = 1048576.0  # out-of-case shift; BIG*rd >= BIG-1 >> all hue edges

# ScalarE sign-counted edges
S_MX = (0.1, 0.2, 0.5, 0.6, 0.7, 0.8, 0.9)   # val: count mx < e (slots 10-16)
S_Q = tuple(10.0 / k for k in range(1, 10))  # sat: count q > e   (slots 17-25)

NSLOT = 26


def _register_custom_ops():
    """Author + register fused DVE ops in the dve_ops registry at runtime."""
    from concourse import dve_ops
    from concourse.dve_spec import (
        C0, C1, C2, One, Spec, Src0, Src1, Zero, _has_src1, lower, maxx, minn,
    )
    from concourse.dve_uop import DveOpSpec
    from operator import add as _add

    if hasattr(dve_ops, "D3X"):
        return dve_ops

    def cref(f):
        def r(in0, in1, c0, c1, c2):
            b = f(in0, in1, c0, c1, c2).astype(np.float32)
            return b, b.reshape(b.shape[0], -1).sum(-1, keepdims=True)
        return r

    _t = Src0 + Src1

    def _mg(v, w):
        return ((v + w) < 0) & (w <= 0)

    def _mb(v, w):
        return (v >= 0) & ((v + w) >= 0)

    defs = [
        # d = max(|v|,|w|,|v+w|) == mx-mn (exact: same fl subtract results)
        ("D3X", Spec(
            body=maxx(maxx(maxx(Src0, Src1), _t), Zero - minn(minn(Src0, Src1), _t)),
            reference=lambda in0, in1, c0, c1, c2: np.maximum(
                np.maximum(np.abs(in0), np.abs(in1)), np.abs(in0 + in1)
            ).astype(np.float32),
        )),
        # s8 = mb + mg from (v,w): mb=(v>=0)&(v+w>=0), mg=(v+w<0)&(w<=0)
        ("S8C", Spec(
            body=((Src0 >= Zero) & (_t >= Zero)) + ((_t < Zero) & (Src1 <= Zero)),
            reference=lambda in0, in1, c0, c1, c2: (
                _mb(in0, in1) + _mg(in0, in1)
            ).astype(np.float32),
        )),
        # v' = v + C0*(1 - mg) with mg=(u>0)&(u+v>=0)   (Src0=u, Src1=v)
        ("VSH2", Spec(
            body=Src1 + C0 * (One - ((Src0 > Zero) & (_t >= Zero))),
            reference=lambda in0, in1, c0, c1, c2: (
                in1 + c0 * (1.0 - ((in0 > 0) & ((in0 + in1) >= 0)))
            ).astype(np.float32),
        )),
        # w' = w + C0*(1 - mb) with mb=(u+w<=0)&(u<=0)  (Src0=u, Src1=w)
        ("WSH2", Spec(
            body=Src1 + C0 * (One - ((_t <= Zero) & (Src0 <= Zero))),
            reference=lambda in0, in1, c0, c1, c2: (
                in1 + c0 * (1.0 - (((in0 + in1) <= 0) & (in0 <= 0)))
            ).astype(np.float32),
        )),
        # dual mult-compare count: accum = #{a*b < C0} + C1*#{a*b < C2}
        ("MULCMP", Spec(
            body=(Src0 * Src1 < C0) + C1 * ((Src0 * Src1) < C2),
            accum=_add, accum_init=Zero,
            reference=cref(lambda a, b, c0, c1, c2: (
                ((a * b).astype(np.float32) < c0)
                + c1 * ((a * b).astype(np.float32) < c2)
            )),
        )),
        # mixed dual count: accum = #{a*b < C0} + C1*#{a < C2}
        ("MULCMPM", Spec(
            body=(Src0 * Src1 < C0) + C1 * (Src0 < C2),
            accum=_add, accum_init=Zero,
            reference=cref(lambda a, b, c0, c1, c2: (
                ((a * b).astype(np.float32) < c0) + c1 * (a < c2)
            )),
        )),
        # dual gt-edge count: accum = #{x > C0} + C1*#{x > C2}
        ("GT2", Spec(
            body=(Src0 > C0) + C1 * (Src0 > C2), accum=_add, accum_init=Zero,
            reference=cref(lambda a, b, c0, c1, c2: (a > c0) + c1 * (a > c2)),
        )),
        # dual edge count: accum = #{x < C0} + C1*#{x < C2}
        ("LT2", Spec(
            body=(Src0 < C0) + C1 * (Src0 < C2),
            accum=_add, accum_init=Zero,
            reference=cref(lambda a, b, c0, c1, c2: (a < c0) + c1 * (a < c2)),
        )),
    ]
    for name, spec in defs:
        row = 1 + len(dve_ops.OPS)
        shas = {}
        for ver in ("v3", "v4"):
            uops = lower(spec, ver=ver)
            shas[ver] = DveOpSpec(
                name=name, opcode=row, uops=uops, rd1_en=_has_src1(spec)
            ).sha(ver)
        op = dve_ops.DveOp(name, spec, False, uops_sha=shas)
        dve_ops.OPS.append(op)
        dve_ops.CUSTOM_DVE_SPECS[name] = spec
        dve_ops._SUB_OPCODE_FOR_NAME[name] = row
        setattr(dve_ops, name, op)
    return dve_ops


def _build():
    dve_ops = _register_custom_ops()
    nc = bacc.Bacc(
        "TRN2", target_bir_lowering=False, debug=False, num_devices=NCORES
    )
    xr = nc.dram_tensor("x_real", [BPC * C * P, F], F32, kind="ExternalInput").ap()
    xf = nc.dram_tensor("x_fake", [BPC * C * P, F], F32, kind="ExternalInput").ap()
    out = nc.dram_tensor("out", [NPAIR * P, ACCW], F32, kind="ExternalOutput").ap()

    SIGN = mybir.ActivationFunctionType.Sign

    with tile.TileContext(nc) as tc:
        with tc.tile_pool(name="main", bufs=2) as io_pool, tc.tile_pool(
            name="tmp", bufs=1
        ) as tmp_pool:
            V, S = nc.vector, nc.scalar
            # bias tiles for ScalarE sign counts (bias = -edge)
            sbias = []
            for idx, e in enumerate(S_MX + S_Q + (0.0, 2.0)):
                bt = tmp_pool.tile([P, 1], F32, tag=f"sb{idx}", name=f"sb{idx}")
                nc.gpsimd.memset(bt[:], -e)
                sbias.append(bt)
            b_mx = sbias[0:7]
            b_q = sbias[7:16]
            b_0, b_2 = sbias[16], sbias[17]

            for pr in range(NPAIR):
                src = xr if pr < NPAIR // 2 else xf

                def plane(j, c):
                    qI = ((pr * 2 + j) % BPC) * C + c
                    return src[qI * P : (qI + 1) * P, :]

                def TP(tag):  # pair tile [P, 2*F] (two images side by side)
                    return tmp_pool.tile([P, 2 * F], F32, tag=tag, name=tag)

                acc = io_pool.tile([P, ACCW], F32, tag="acc")
                if pr == NPAIR - 1:
                    nc.vector.memset(acc[:, 17:21], 0.0)
                vP = TP("vP")
                wP = TP("wP")
                uP = TP("uP")
                mxP = TP("mxP")
                qP = TP("qP")
                rdP = TP("rdP")
                s8P = TP("s8P")
                scrP = TP("scrP")   # doubles as d
                scr2 = tmp_pool.tile(
                    [P, 2 * F], mybir.dt.bfloat16, tag="scr2", name="scr2"
                )
                m1 = tmp_pool.tile([P, F], F32, tag="m1", name="m1", bufs=2)

                for j in range(2):
                    r = io_pool.tile([P, F], F32, tag="r", name="r")
                    g = io_pool.tile([P, F], F32, tag="g", name="g")
                    bl = io_pool.tile([P, F], F32, tag="bl", name="bl")
                    nc.sync.dma_start(bl[:], plane(j, 2))
                    nc.sync.dma_start(r[:], plane(j, 0))
                    nc.sync.dma_start(g[:], plane(j, 1))
                    sl = slice(j * F, (j + 1) * F)
                    V.tensor_tensor(vP[:, sl], bl[:], r[:], AF.subtract)
                    V.tensor_tensor(wP[:, sl], r[:], g[:], AF.subtract)
                    V.tensor_tensor(m1[:], r[:], g[:], AF.max)
                    V.tensor_tensor(uP[:, sl], g[:], bl[:], AF.subtract)
                    V.tensor_tensor(mxP[:, sl], m1[:], bl[:], AF.max)

                # pair-level fused ops
                V._custom_dve(dve_ops.D3X, out=scrP[:], in0=vP[:], in1=wP[:])
                V.reciprocal_approx_fast(rdP[:], scrP[:])
                V._custom_dve(dve_ops.S8C, out=s8P[:], in0=vP[:], in1=wP[:])
                # in-place shifts; masks recomputed from original u, which is
                # only overwritten by the final STT
                V._custom_dve(dve_ops.WSH2, out=wP[:], in0=uP[:], in1=wP[:], s0=BIG)
                V._custom_dve(dve_ops.VSH2, out=vP[:], in0=uP[:], in1=vP[:], s0=BIG)
                V.scalar_tensor_tensor(uP[:], s8P[:], BIG, uP[:], AF.mult, AF.add)
                V.tensor_tensor(qP[:], mxP[:], rdP[:], AF.mult)

                # --- ScalarE sign counts (mx first, then u'/v', then q) ---
                for k in range(7):
                    S.activation(scr2[:], mxP[:], SIGN, bias=b_mx[k][:],
                                 accum_out=acc[:, 10 + k : 11 + k])

                # --- VectorE dual counts (pair-level) ---
                def mc(op, src0, src1, e1, e2, slot):
                    V._custom_dve(op, out=scrP[:], in0=src0[:], in1=src1[:],
                                  s0=e1, s1=PACK, imm2=e2,
                                  accum_out=acc[:, slot : slot + 1])

                mc(dve_ops.MULCMP, uP, rdP, -0.6, 0.6, 0)
                mc(dve_ops.MULCMP, vP, rdP, -0.8, -0.2, 1)
                mc(dve_ops.MULCMP, vP, rdP, 0.4, 1.0, 2)
                mc(dve_ops.MULCMP, wP, rdP, -0.4, 0.2, 3)
                # slot4: {w'rd < 0.8} + PACK*{w' < 2}  (NB(0.8) + Bt)
                mc(dve_ops.MULCMPM, wP, rdP, 0.8, 2.0, 4)
                V._custom_dve(dve_ops.LT2, out=scrP[:], in0=mxP[:], s0=0.3,
                              s1=PACK, imm2=0.4, accum_out=acc[:, 5:6])

                # --- remaining ScalarE sign counts ---
                S.activation(scr2[:], uP[:], SIGN, bias=b_0[:],
                             accum_out=acc[:, 7:8])
                S.activation(scr2[:], vP[:], SIGN, bias=b_2[:],
                             accum_out=acc[:, 8:9])
                # last pair: move sat edges k=1..4 to VectorE so ScalarE
                # is not the pipeline tail (slot 26: k=1,2; slot 27: k=3,4)
                qk0 = 4 if pr == NPAIR - 1 else 0
                if qk0:
                    V._custom_dve(dve_ops.GT2, out=scrP[:], in0=qP[:],
                                  s0=10.0, s1=PACK, imm2=5.0,
                                  accum_out=acc[:, 26:27])
                    V._custom_dve(dve_ops.GT2, out=scrP[:], in0=qP[:],
                                  s0=10.0 / 3.0, s1=PACK, imm2=2.5,
                                  accum_out=acc[:, 27:28])
                for k in range(qk0, 9):
                    S.activation(scr2[:], qP[:], SIGN, bias=b_q[k][:],
                                 accum_out=acc[:, 17 + k : 18 + k])

                nc.sync.dma_start(out[pr * P : (pr + 1) * P, 0:ACCW], acc[:, :])

    nc.compile()
    return nc


def _register_ntff_hook():
    """Register the axon NTFF profiling hook; keep artifacts local."""
    import types

    import antenv

    if "antenv.axon_hooks" not in sys.modules:
        mod = types.ModuleType("antenv.axon_hooks")
        holder = [None]
        mod.set_axon_ntff_profile_hook = lambda h: holder.__setitem__(0, h)
        mod.get_axon_ntff_profile_hook = lambda: holder[0]
        sys.modules["antenv.axon_hooks"] = mod
        antenv.axon_hooks = mod
    from antenv import axon_hooks

    if axon_hooks.get_axon_ntff_profile_hook() is None:
        from trn_agent_boot.trn_boot import _ntff_profile_via_ctypes

        axon_hooks.set_axon_ntff_profile_hook(
            _ntff_profile_via_ctypes("/opt/axon/libaxon_pjrt.so")
        )
    bass_utils.upload_artifacts = lambda tmpdir: tmpdir


def _get_nc():
    if "nc" not in _CACHE:
        _CACHE["nc"] = _build()
    return _CACHE["nc"]


def kernel(x_real: np.ndarray, x_fake: np.ndarray) -> np.ndarray:
    global LAST_EXEC_NS
    nc = _get_nc()

    in_maps = []
    for c in range(NCORES):
        sl = slice(c * BPC, (c + 1) * BPC)
        in_maps.append(
            {
                "x_real": np.ascontiguousarray(x_real[sl]).reshape(BPC * C * P, F),
                "x_fake": np.ascontiguousarray(x_fake[sl]).reshape(BPC * C * P, F),
            }
        )

    import os

    trace = bool(int(os.environ.get("KERNEL_TRACE", "0")))
    if trace:
        _register_ntff_hook()
    res = bass_utils.run_bass_kernel_spmd(
        nc, in_maps, core_ids=list(range(NCORES)), trace=trace
    )
    LAST_EXEC_NS = res.exec_time_ns
    _CACHE["last_res"] = res

    # ---- host decode ----
    # slots: 0 u'rd dual(-0.6,0.6) | 1 v'rd dual(-0.8,-0.2) | 2 v'rd dual(0.4,1.0)
    #        3 w'rd dual(-0.4,0.2) | 4 {w'rd<0.8}+P*{w'<2} | 5 mx dual(0.3,0.4)
    #        7 sign(u') | 8 sign(v'-2) | 10-16 sign(mx-e), e in S_MX
    #        17-25 sign(q-10/k), k=1..9
    slots = np.zeros((2, NSLOT), np.float64)
    duals = np.zeros((2, 6, 2), np.float64)  # per-row unpacked dual slots 0..5
    sat_direct = np.zeros(4, np.float64)     # last-pair sat counts k=1..4
    half = NPAIR // 2
    for core_out in res.results:
        o = np.asarray(core_out["out"]).reshape(NPAIR, P, ACCW)
        slots[0] += o[:half, :, :NSLOT].sum(axis=(0, 1))
        slots[1] += o[half:, :, :NSLOT].sum(axis=(0, 1))
        pk = o[:, :, 0:6].astype(np.int64)  # exact ints in f32
        lo, hi = pk % int(PACK), pk // int(PACK)
        for t_idx, sl in ((0, slice(0, half)), (1, slice(half, NPAIR))):
            duals[t_idx, :, 0] += lo[sl].sum(axis=(0, 1))
            duals[t_idx, :, 1] += hi[sl].sum(axis=(0, 1))
        pk2 = o[NPAIR - 1, :, 26:28].astype(np.int64)
        sat_direct[0::2] += (pk2 % int(PACK)).sum(axis=0)
        sat_direct[1::2] += (pk2 // int(PACK)).sum(axis=0)

    N = float(NPIX)  # pixels per source across all cores

    C_lt = np.zeros((2, 3, 9), np.float64)
    for t in range(2):
        NA_lo, NA_hi = duals[t, 0]                 # NA(-0.6), NA(0.6)
        NG_m8, NG_m2 = duals[t, 1]
        NG_04, NG_10 = duals[t, 2]
        NB_m4, NB_02 = duals[t, 3]
        NB_08, Bt = duals[t, 4]                    # NB(0.8), caseB total
        NEG = (N - slots[t, 7]) / 2.0              # #{u' < 0}
        Gt = (N - slots[t, 8]) / 2.0               # caseG total
        R = N - Gt - Bt
        # hue cumulative at 0.6k, k=1..9
        C_lt[t, 0, 0] = NA_hi - NEG
        for j, ng in enumerate((NG_m8, NG_m2, NG_04, NG_10)):
            C_lt[t, 0, 1 + j] = (R - NEG) + ng
        for j, nb in enumerate((NB_m4, NB_02, NB_08)):
            C_lt[t, 0, 5 + j] = (R - NEG) + Gt + nb
        C_lt[t, 0, 8] = (N - NEG) + NA_lo
        # val cumulative at 0.1k: 0.1,0.2 signs; 0.3,0.4 dual; 0.5-0.9 signs
        C_lt[t, 2, 0] = (N - slots[t, 10]) / 2.0
        C_lt[t, 2, 1] = (N - slots[t, 11]) / 2.0
        C_lt[t, 2, 2:4] = duals[t, 5]
        for k in range(5):
            C_lt[t, 2, 4 + k] = (N - slots[t, 12 + k]) / 2.0
        # sat cumulative: C(0.1k) = #{q > 10/k} = (N + sign_sum)/2
        # fake source: sat k=1..4 of the last pair were counted directly on
        # VectorE, so the sign part covers only the first pair (N/2 pixels)
        for k in range(9):
            if t == 1 and k < 4:
                C_lt[t, 1, k] = (N / 2.0 + slots[t, 17 + k]) / 2.0 + sat_direct[k]
            else:
                C_lt[t, 1, k] = (N + slots[t, 17 + k]) / 2.0

    hist = np.zeros((2, 3, 10), np.float64)
    hist[:, :, 0] = C_lt[:, :, 0]
    hist[:, :, 1:9] = C_lt[:, :, 1:] - C_lt[:, :, :-1]
    hist[:, :, 9] = N - C_lt[:, :, 8]

    dmean = np.abs(hist[0] - hist[1]).mean(axis=1)   # [3] = h, s, v
    loss = ALPHA * dmean[0] + BETA * dmean[1] + GAMMA * dmean[2]
    return np.asarray(loss, dtype=np.float32)
